# revision 9
# baseline (speedup 1.0000x reference)
"""GPT-J attention (B=2, S=2048, D=4096, H=16, HD=256, ROT=64, causal) on 8 TRN2 NeuronCores.

Sharding: DP over batch (2 groups of 4 cores) x TP over heads (4 heads/core).

v2 restructure vs the session-1 baseline:
- Weights stream once per 1024-token half (not once per 512-token tile):
  x / q / k / v are SBUF-resident, cutting weight DMA 134MB -> ~58MB and
  keeping the PE warm (no HAM oscillation from DMA stalls).
- Attention is software-pipelined 2 jt-blocks deep; softmax tails lag one
  head so the reciprocal never stalls the PE.
- gpsimd.partition_broadcast replaces the PE broadcast-matmul + copy;
  rope runs in bf16 with 2 of 5 elementwise ops on the idle GpSimd.
- Diagonal causal blocks compute only columns [128a:512] (column trim).
- Schedule: proj(half0) -> attn/outproj tiles 1,0 -> proj(half1) ->
  attn/outproj tiles 3,2; per-tile ReduceScatter overlaps later compute;
  the last tile's out-proj is split into two row-halves with staggered
  ReduceScatters to shrink the exposed tail.

Compute dtype: bf16 on the TensorEngine (f32 PSUM accumulation).
"""

import numpy as np
import ml_dtypes

import concourse.bass as bass
import concourse.mybir as mybir
import concourse.tile as tile
from concourse import bacc
from concourse.bass_utils import run_bass_kernel_spmd

B, S, D = 2, 2048, 4096
H, HD, ROT = 16, 256, 64
NCORES, DP, TP = 8, 2, 4
NH = H // TP              # 4 local heads
QL = NH * HD              # 1024 local q/k/v dims
T = S                     # local tokens (one batch per DP group)
NT = T // 512             # 4 token tiles of 512
KC = D // 128             # 32 contraction chunks
WQR = QL // 256           # 4 ob-pair passes for q/k (= one head each)
BF16 = mybir.dt.bfloat16
F32 = mybir.dt.float32
INV_SCALE = 1.0 / 16.0    # 1/sqrt(HD)
GROUPS = [[0, 1, 2, 3], [4, 5, 6, 7]]
NPBF16 = ml_dtypes.bfloat16
Copy = mybir.ActivationFunctionType.Copy
Exp = mybir.ActivationFunctionType.Exp
ds, ts = bass.ds, bass.ts


def build_nc():
    nc = bacc.Bacc("TRN2", target_bir_lowering=False, num_devices=NCORES)
    xT = nc.declare_dram_parameter("xT", [D, T], BF16, isOutput=False)
    wq_r = nc.declare_dram_parameter("wq_r", [WQR, KC, 128, 256], BF16, isOutput=False)
    wk_r = nc.declare_dram_parameter("wk_r", [WQR, KC, 128, 256], BF16, isOutput=False)
    wv_r = nc.declare_dram_parameter("wv_r", [2, KC, 128, 512], BF16, isOutput=False)
    wo_p = nc.declare_dram_parameter("wo_p", [D // 256, QL // 128, 128, 256], BF16, isOutput=False)
    # trig rows: [cos_even(32); cos_odd(32); sin_even(32); sin_odd(32)]
    trig = nc.declare_dram_parameter("trig", [128, T], BF16, isOutput=False)
    tri = nc.declare_dram_parameter("tri", [128, 128], BF16, isOutput=False)
    outc = nc.declare_dram_parameter("out_chunk", [QL, T], BF16, isOutput=True)

    # RS staging. Tiles 1, 0, 3 use full [D,512] buffers; tile 2 (the tail)
    # is split into two row-halves: parts2[i][r*512:(r+1)*512] holds rank r's
    # D-rows [1024r + 512i, 1024r + 512(i+1)).
    parts = {tt: nc.dram_tensor(f"part{tt}", [D, 512], BF16) for tt in (0, 1, 3)}
    rss = {tt: nc.dram_tensor(f"rs{tt}", [QL, 512], BF16) for tt in (0, 1, 3)}
    parts2 = [nc.dram_tensor(f"part2_{i}", [D // 2, 512], BF16) for i in range(2)]
    rss2 = [nc.dram_tensor(f"rs2_{i}", [QL // 2, 512], BF16) for i in range(2)]

    with tile.TileContext(nc) as tc:
        with (
            tc.tile_pool(name="singles", bufs=1) as singles,
            tc.tile_pool(name="xp", bufs=2) as xp,
            tc.tile_pool(name="kp", bufs=1) as kp,
            tc.tile_pool(name="qp", bufs=2) as qp,
            tc.tile_pool(name="vp", bufs=1) as vp,
            tc.tile_pool(name="wqk", bufs=6) as wqk,
            tc.tile_pool(name="wvp", bufs=4) as wvp,
            tc.tile_pool(name="wop", bufs=6) as wop,
            tc.tile_pool(name="etp", bufs=3) as etp,
            tc.tile_pool(name="rt", bufs=4) as rtp,
            tc.tile_pool(name="rcpp", bufs=2) as rcpp,
            tc.tile_pool(name="bcp", bufs=2) as bcp,
            tc.tile_pool(name="atp", bufs=10) as atp,
            tc.tile_pool(name="osb", bufs=4) as osbp,
            tc.tile_pool(name="ps", bufs=2, space="PSUM") as psp,
        ):
            # --- constants ---
            trig_sb = singles.tile([128, T], BF16, name="trig_sb")
            nc.sync.dma_start(out=trig_sb, in_=trig[:, :])
            tri_sb = singles.tile([128, 128], BF16, name="tri_sb")
            nc.sync.dma_start(out=tri_sb, in_=tri[:, :])
            ones128 = singles.tile([128, 1], BF16, name="ones128")
            nc.vector.memset(ones128, 1.0)

            x = [None] * NT

            def load_x2(ta, tb):
                """Load two x tiles with ci-interleaved sub-DMAs so the first
                contraction chunks of both tiles land early."""
                for tt in (ta, tb):
                    x[tt] = xp.tile([128, KC, 512], BF16, tag="x", name=f"x{tt}")
                for ci in range(4):
                    for tt in (ta, tb):
                        nc.sync.dma_start(
                            out=x[tt][:, ds(8 * ci, 8), :],
                            in_=xT[ds(1024 * ci, 1024), ts(tt, 512)].rearrange(
                                "(c p) t -> p c t", p=128),
                        )

            kt = [None] * NT          # kt[tt]: [128, 8, 512] bf16 (dim-major)
            qt = [None] * NT          # qt[tt]: same shape, pool-rotated (bufs=2)
            vt = [None] * (NT * 4)    # vt[j]: [128, 1024] bf16 (token-major)
            atiles = {}               # (tt, b) -> [128, 512] bf16 attn out

            PTAGS = ("pa", "pb", "pc", "pd")

            def rope_evac(ps, dst, tt, par):
                """Evacuate psum [128,512] (rows 0-31 rot-even, 32-63 rot-odd,
                64-127 pass) into bf16 dst view, applying the GPT-J rotation.
                Multiplies read PSUM (exempt from the SB/SB same-base-partition
                rule); the final sub/add see partition-aligned SBUF operands."""
                tcols = ts(tt, 512)
                nc.scalar.activation(out=dst[64:128, :], in_=ps[64:128, :], func=Copy)
                tc_eo = rtp.tile([64, 512], BF16, tag="rt", name="tc_eo")
                nc.vector.tensor_mul(out=tc_eo, in0=ps[0:64, :], in1=trig_sb[0:64, tcols])
                ts_eo = rtp.tile([64, 512], BF16, tag="rt", name="ts_eo")
                nc.vector.tensor_mul(out=ts_eo[0:32, :], in0=ps[32:64, :], in1=trig_sb[64:96, tcols])
                nc.vector.tensor_mul(out=ts_eo[32:64, :], in0=ps[0:32, :], in1=trig_sb[96:128, tcols])
                nc.gpsimd.tensor_sub(out=dst[0:32, :], in0=tc_eo[0:32, :], in1=ts_eo[0:32, :])
                nc.gpsimd.tensor_add(out=dst[32:64, :], in0=tc_eo[32:64, :], in1=ts_eo[32:64, :])

            def proj_pass(half, wr, qr, is_q, first=False):
                """ob-pair (head qr) q/k projection over this half's 1024 tokens.
                first=True: weight loads all on the scalar queue (sync is busy
                streaming x at kernel start)."""
                t0, t1 = 2 * half, 2 * half + 1
                ps = [psp.tile([128, 512], F32, tag=PTAGS[i], name=f"pp{i}")
                      for i in range(4)]
                for dc in range(KC):
                    w_t = wqk.tile([128, 256], BF16, tag="w", name="w_t")
                    eng = nc.scalar if first else (nc.sync if dc % 2 == 0 else nc.scalar)
                    eng.dma_start(out=w_t, in_=wr[qr, dc, :, :])
                    st, sp = dc == 0, dc == KC - 1
                    nc.tensor.matmul(ps[0], w_t[:, 0:128], x[t0][:, dc, :], start=st, stop=sp)
                    nc.tensor.matmul(ps[1], w_t[:, 0:128], x[t1][:, dc, :], start=st, stop=sp)
                    nc.tensor.matmul(ps[2], w_t[:, 128:256], x[t0][:, dc, :], start=st, stop=sp)
                    nc.tensor.matmul(ps[3], w_t[:, 128:256], x[t1][:, dc, :], start=st, stop=sp)
                for ti, tt in ((0, t0), (1, t1)):
                    dstbuf = qt[tt] if is_q else kt[tt]
                    rope_evac(ps[ti], dstbuf[:, 2 * qr, :], tt, ti)
                    eng = nc.scalar if ti == 0 else nc.vector
                    if ti == 0:
                        nc.scalar.activation(out=dstbuf[:, 2 * qr + 1, :], in_=ps[2 + ti], func=Copy)
                    else:
                        nc.vector.tensor_copy(dstbuf[:, 2 * qr + 1, :], ps[2 + ti])

            def v_group(half, vh):
                """v projection: 8 token-blocks (this half) x one dv-half of 512."""
                pv = [psp.tile([128, 512], F32, tag=PTAGS[i // 2], name=f"pv{i}")
                      for i in range(8)]
                for dc in range(KC):
                    wv_t = wvp.tile([128, 512], BF16, tag="wv", name="wv_t")
                    eng = nc.sync if dc % 2 == 0 else nc.scalar
                    eng.dma_start(out=wv_t, in_=wv_r[vh, dc, :, :])
                    st, sp = dc == 0, dc == KC - 1
                    for tb in range(8):
                        tt = 2 * half + tb // 4
                        nc.tensor.matmul(pv[tb], x[tt][:, dc, ds(128 * (tb % 4), 128)],
                                         wv_t, start=st, stop=sp)
                for tb in range(8):
                    j = 8 * half + tb
                    if vh == 0:
                        vt[j] = vp.tile([128, 1024], BF16, tag=f"v{j}", name=f"vt{j}")
                    if tb % 2 == 0:
                        nc.vector.tensor_copy(vt[j][:, ds(512 * vh, 512)], pv[tb])
                    else:
                        nc.scalar.activation(out=vt[j][:, ds(512 * vh, 512)], in_=pv[tb], func=Copy)

            def proj_half(half):
                t0, t1 = 2 * half, 2 * half + 1
                for tt in (t0, t1):
                    kt[tt] = kp.tile([128, 8, 512], BF16, tag=f"k{tt}", name=f"kt{tt}")
                    qt[tt] = qp.tile([128, 8, 512], BF16, tag="q", name=f"qt{tt}")
                for wi, (wr, is_q) in enumerate(((wq_r, True), (wk_r, False))):
                    for qr in range(WQR):
                        proj_pass(half, wr, qr, is_q,
                                  first=(half == 0 and wi == 0 and qr == 0))
                for vh in range(2):
                    v_group(half, vh)

            def attn_tile(tt):
                njt = 4 * tt + 4
                tails = []

                def emit_sc(h, jt):
                    a = jt - 4 * tt
                    c0 = 128 * a if a > 0 else 0
                    ktt, kj = jt // 4, jt % 4
                    sc = psp.tile([128, 512], F32, tag="pd", name="sc")
                    nc.tensor.matmul(sc[:, c0:], kt[ktt][:, 2 * h, ds(128 * kj, 128)],
                                     qt[tt][:, 2 * h, c0:], start=True, stop=False)
                    nc.tensor.matmul(sc[:, c0:], kt[ktt][:, 2 * h + 1, ds(128 * kj, 128)],
                                     qt[tt][:, 2 * h + 1, c0:], start=False, stop=True)
                    et = etp.tile([128, 512], BF16, tag="et", name="et")
                    nc.scalar.activation(out=et[:, c0:], in_=sc[:, c0:], func=Exp,
                                         scale=INV_SCALE)
                    if a >= 0:
                        nc.vector.tensor_mul(out=et[:, ds(c0, 128)], in0=et[:, ds(c0, 128)],
                                             in1=tri_sb)
                    return et, c0

                for h in range(NH):
                    av0 = psp.tile([128, 512], F32, tag="pa", name="av0")
                    av1 = psp.tile([128, 512], F32, tag="pb", name="av1")
                    sums = psp.tile([1, 512], F32, tag="pc", name="sums")
                    pend = [emit_sc(h, 0)]
                    if njt > 1:
                        pend.append(emit_sc(h, 1))
                    for jt in range(njt):
                        if jt + 2 < njt:
                            pend.append(emit_sc(h, jt + 2))
                        et, c0 = pend.pop(0)
                        st, sp = jt == 0, jt == njt - 1
                        nc.tensor.matmul(sums[:, c0:], ones128, et[:, c0:], start=st, stop=sp)
                        nc.tensor.matmul(av0[:, c0:], vt[jt][:, ds(256 * h, 128)],
                                         et[:, c0:], start=st, stop=sp)
                        nc.tensor.matmul(av1[:, c0:], vt[jt][:, ds(256 * h + 128, 128)],
                                         et[:, c0:], start=st, stop=sp)
                        if jt == 2 and tails:
                            tails.pop(0)()

                    def tail(h=h, av0=av0, av1=av1, sums=sums):
                        rcp = rcpp.tile([1, 512], F32, tag="rcp", name="rcp")
                        scr = rcpp.tile([1, 512], F32, tag="scr", name="scr")
                        nc.vector.reciprocal_approx_accurate(out=rcp, in_=sums, scratch=scr)
                        bc = bcp.tile([128, 512], F32, tag="bc", name="bc")
                        nc.gpsimd.partition_broadcast(bc, rcp, channels=128)
                        a0 = atp.tile([128, 512], BF16, tag="at", name="a0")
                        a1 = atp.tile([128, 512], BF16, tag="at", name="a1")
                        nc.vector.tensor_mul(out=a0, in0=av0, in1=bc)
                        nc.vector.tensor_mul(out=a1, in0=av1, in1=bc)
                        atiles[(tt, 2 * h)], atiles[(tt, 2 * h + 1)] = a0, a1
                    tails.append(tail)
                while tails:
                    tails.pop(0)()

            def oproj_ctp(tt, ctp, ci, dest_rows):
                """One 256-out-dim column-pair of the out projection for tile tt.
                dest_rows: (tensor, row_offset) for the 256 output rows."""
                tga, tgb = (PTAGS[0], PTAGS[1]) if ci % 2 == 0 else (PTAGS[2], PTAGS[3])
                po0 = psp.tile([128, 512], F32, tag=tga, name="po0")
                po1 = psp.tile([128, 512], F32, tag=tgb, name="po1")
                for dc in range(QL // 128):
                    wo_t = wop.tile([128, 256], BF16, tag="wo", name="wo_t")
                    eng = nc.sync if dc % 2 == 0 else nc.scalar
                    eng.dma_start(out=wo_t, in_=wo_p[ctp, dc, :, :])
                    st, sp = dc == 0, dc == QL // 128 - 1
                    nc.tensor.matmul(po0, wo_t[:, 0:128], atiles[(tt, dc)], start=st, stop=sp)
                    nc.tensor.matmul(po1, wo_t[:, 128:256], atiles[(tt, dc)], start=st, stop=sp)
                dtensor, roff = dest_rows
                for hf, po in ((0, po0), (1, po1)):
                    o_sb = osbp.tile([128, 512], BF16, tag="o", name="o_sb")
                    if hf == 0:
                        nc.vector.tensor_copy(o_sb, po)
                    else:
                        nc.scalar.activation(out=o_sb, in_=po, func=Copy)
                    nc.sync.dma_start(out=dtensor[ds(roff + 128 * hf, 128), :], in_=o_sb)

            def oproj_tile(tt):
                for ci, ctp in enumerate(range(D // 256)):
                    oproj_ctp(tt, ctp, ci, (parts[tt], 256 * ctp))
                nc.gpsimd.collective_compute(
                    "ReduceScatter", mybir.AluOpType.add, replica_groups=GROUPS,
                    ins=[parts[tt][:]], outs=[rss[tt][:]],
                )
                # outc DMA deferred to program end: a sync-queue DMA that waits
                # on the collective would block all later weight loads (74us
                # PE stall measured at the proj_half(1) boundary).

            def oproj_tile_split(tt):
                # row-half i: per rank r the D-rows [1024r+512i, 1024r+512(i+1)),
                # i.e. ctp where (256*ctp % 1024)//512 == i.
                for i in range(2):
                    ctps = [c for c in range(D // 256) if (256 * c % 1024) // 512 == i]
                    for ci, ctp in enumerate(ctps):
                        r = 256 * ctp // 1024
                        roff = 512 * r + (256 * ctp % 1024) - 512 * i
                        oproj_ctp(tt, ctp, ci, (parts2[i], roff))
                    nc.gpsimd.collective_compute(
                        "ReduceScatter", mybir.AluOpType.add, replica_groups=GROUPS,
                        ins=[parts2[i][:]], outs=[rss2[i][:]],
                    )

            def flush_outc(split_tt):
                for tt in (1, 0, 3):
                    nc.sync.dma_start(out=outc[:, ts(tt, 512)], in_=rss[tt][:])
                for i in range(2):
                    nc.sync.dma_start(out=outc[ds(512 * i, 512), ts(split_tt, 512)],
                                      in_=rss2[i][:])

            # ---------------- schedule ----------------
            load_x2(0, 1)
            proj_half(0)
            load_x2(2, 3)
            attn_tile(1)
            oproj_tile(1)
            attn_tile(0)
            oproj_tile(0)
            proj_half(1)
            attn_tile(3)
            oproj_tile(3)
            attn_tile(2)
            oproj_tile_split(2)
            flush_outc(2)

    nc.compile()
    return nc


_ROT_PERM = np.concatenate([np.arange(0, ROT, 2), np.arange(1, ROT, 2), np.arange(ROT, HD)])


def make_in_maps(hidden_states, sin, cos, Wq, Wk, Wv, Wo):
    hidden_states = np.asarray(hidden_states, dtype=np.float32)
    sin = np.asarray(sin, dtype=np.float32)
    cos = np.asarray(cos, dtype=np.float32)
    Wq, Wk, Wv, Wo = (np.asarray(w, dtype=np.float32) for w in (Wq, Wk, Wv, Wo))

    ce = cos[0, :, 0, 0::2].T  # (32, S)
    co = cos[0, :, 0, 1::2].T
    se = sin[0, :, 0, 0::2].T
    so = sin[0, :, 0, 1::2].T
    trig = np.ascontiguousarray(np.concatenate([ce, co, se, so], axis=0)).astype(NPBF16)

    j = np.arange(128)[:, None]
    i = np.arange(128)[None, :]
    tri = (j <= i).astype(NPBF16)  # (128, 128) lower-left valid (k_off <= q_off)

    def pack_qk(w_loc_perm_T):
        # (D, QL) -> (WQR, KC, 128, 256)
        w4 = w_loc_perm_T.reshape(KC, 128, WQR, 256)
        return np.ascontiguousarray(w4.transpose(2, 0, 1, 3))

    def pack_v(wvT_loc):
        w4 = wvT_loc.reshape(KC, 128, 2, 512)
        return np.ascontiguousarray(w4.transpose(2, 0, 1, 3))

    in_maps = []
    for c in range(NCORES):
        dp, tp = divmod(c, TP)
        xT = np.ascontiguousarray(hidden_states[dp].T).astype(NPBF16)
        rows = np.arange(QL * tp, QL * (tp + 1))
        perm_rows = np.concatenate([QL * tp + 256 * h + _ROT_PERM for h in range(NH)])
        wqT = np.ascontiguousarray(Wq[perm_rows].T).astype(NPBF16)  # (D, QL)
        wkT = np.ascontiguousarray(Wk[perm_rows].T).astype(NPBF16)
        wvT = np.ascontiguousarray(Wv[rows].T).astype(NPBF16)
        woT = Wo[:, rows].T.astype(NPBF16)                          # (QL, D)
        wo_p = np.ascontiguousarray(
            woT.reshape(QL // 128, 128, D // 256, 256).transpose(2, 0, 1, 3))
        in_maps.append({
            "xT": xT,
            "wq_r": pack_qk(wqT),
            "wk_r": pack_qk(wkT),
            "wv_r": pack_v(wvT),
            "wo_p": wo_p,
            "trig": trig,
            "tri": tri,
        })
    return in_maps


def assemble_output(results):
    out = np.empty((B, S, D), dtype=np.float32)
    for dp in range(DP):
        outT = np.concatenate(
            [results[dp * TP + tp]["out_chunk"].astype(np.float32) for tp in range(TP)],
            axis=0,
        )  # (D, S)
        out[dp] = outT.T
    return out


_NC_CACHE = None


def _get_nc():
    global _NC_CACHE
    if _NC_CACHE is None:
        _NC_CACHE = build_nc()
    return _NC_CACHE


def run(trace=False, **inputs):
    in_maps = make_in_maps(**inputs)
    nc = _get_nc()
    r = run_bass_kernel_spmd(nc, in_maps, core_ids=list(range(NCORES)), trace=trace)
    return assemble_output(r.results), r


def kernel(**inputs):
    out, _ = run(trace=False, **inputs)
    return out


# revision 12
# speedup vs baseline: 1.0240x; 1.0240x over previous
"""GPT-J attention (B=2, S=2048, D=4096, H=16, HD=256, ROT=64, causal) on 8 TRN2 NeuronCores.

Sharding: DP over batch (2 groups of 4 cores) x TP over heads (4 heads/core).

v2 restructure vs the session-1 baseline:
- Weights stream once per 1024-token half (not once per 512-token tile):
  x / q / k / v are SBUF-resident, cutting weight DMA 134MB -> ~58MB and
  keeping the PE warm (no HAM oscillation from DMA stalls).
- Attention is software-pipelined 2 jt-blocks deep; softmax tails lag one
  head so the reciprocal never stalls the PE.
- gpsimd.partition_broadcast replaces the PE broadcast-matmul + copy;
  rope runs in bf16 with 2 of 5 elementwise ops on the idle GpSimd.
- Diagonal causal blocks compute only columns [128a:512] (column trim).
- Schedule: proj(half0) -> attn/outproj tiles 1,0 -> proj(half1) ->
  attn/outproj tiles 3,2; per-tile ReduceScatter overlaps later compute;
  the last tile's out-proj is split into two row-halves with staggered
  ReduceScatters to shrink the exposed tail.

Compute dtype: bf16 on the TensorEngine (f32 PSUM accumulation).
"""

import numpy as np
import ml_dtypes

import concourse.bass as bass
import concourse.mybir as mybir
import concourse.tile as tile
from concourse.tile import add_dep_helper
from concourse import bacc
from concourse.bass_utils import run_bass_kernel_spmd

B, S, D = 2, 2048, 4096
H, HD, ROT = 16, 256, 64
NCORES, DP, TP = 8, 2, 4
NH = H // TP              # 4 local heads
QL = NH * HD              # 1024 local q/k/v dims
T = S                     # local tokens (one batch per DP group)
NT = T // 512             # 4 token tiles of 512
KC = D // 128             # 32 contraction chunks
WQR = QL // 256           # 4 ob-pair passes for q/k (= one head each)
BF16 = mybir.dt.bfloat16
F32 = mybir.dt.float32
INV_SCALE = 1.0 / 16.0    # 1/sqrt(HD)
GROUPS = [[0, 1, 2, 3], [4, 5, 6, 7]]
NPBF16 = ml_dtypes.bfloat16
Copy = mybir.ActivationFunctionType.Copy
Exp = mybir.ActivationFunctionType.Exp
ds, ts = bass.ds, bass.ts


def build_nc():
    nc = bacc.Bacc("TRN2", target_bir_lowering=False, num_devices=NCORES)
    xT = nc.declare_dram_parameter("xT", [D, T], BF16, isOutput=False)
    wq_r = nc.declare_dram_parameter("wq_r", [WQR, KC, 128, 256], BF16, isOutput=False)
    wk_r = nc.declare_dram_parameter("wk_r", [WQR, KC, 128, 256], BF16, isOutput=False)
    wv_r = nc.declare_dram_parameter("wv_r", [2, KC, 128, 512], BF16, isOutput=False)
    wo_p = nc.declare_dram_parameter("wo_p", [D // 256, QL // 128, 128, 256], BF16, isOutput=False)
    # trig rows: [cos_even(32); cos_odd(32); sin_even(32); sin_odd(32)]
    trig = nc.declare_dram_parameter("trig", [128, T], BF16, isOutput=False)
    tri = nc.declare_dram_parameter("tri", [128, 128], BF16, isOutput=False)
    outc = nc.declare_dram_parameter("out_chunk", [QL, T], BF16, isOutput=True)

    # RS staging. Tiles 1, 0, 3 use full [D,512] buffers; tile 2 (the tail)
    # is split into two row-halves: parts2[i][r*512:(r+1)*512] holds rank r's
    # D-rows [1024r + 512i, 1024r + 512(i+1)).
    parts = {tt: nc.dram_tensor(f"part{tt}", [D, 512], BF16) for tt in (0, 1, 3)}
    rss = {tt: nc.dram_tensor(f"rs{tt}", [QL, 512], BF16) for tt in (0, 1, 3)}
    parts2 = [nc.dram_tensor(f"part2_{i}", [D // 2, 512], BF16) for i in range(2)]
    rss2 = [nc.dram_tensor(f"rs2_{i}", [QL // 2, 512], BF16) for i in range(2)]

    with tile.TileContext(nc) as tc:
        with (
            tc.tile_pool(name="singles", bufs=1) as singles,
            tc.tile_pool(name="xp", bufs=2) as xp,
            tc.tile_pool(name="kp", bufs=1) as kp,
            tc.tile_pool(name="qp", bufs=2) as qp,
            tc.tile_pool(name="vp", bufs=1) as vp,
            tc.tile_pool(name="wqk", bufs=6) as wqk,
            tc.tile_pool(name="wvp", bufs=4) as wvp,
            tc.tile_pool(name="wop", bufs=6) as wop,
            tc.tile_pool(name="etp", bufs=3) as etp,
            tc.tile_pool(name="rt", bufs=4) as rtp,
            tc.tile_pool(name="rcpp", bufs=2) as rcpp,
            tc.tile_pool(name="bcp", bufs=2) as bcp,
            tc.tile_pool(name="atp", bufs=10) as atp,
            tc.tile_pool(name="osb", bufs=4) as osbp,
            tc.tile_pool(name="ps", bufs=2, space="PSUM") as psp,
        ):
            # --- constants ---
            trig_sb = singles.tile([128, T], BF16, name="trig_sb")
            nc.sync.dma_start(out=trig_sb, in_=trig[:, :])
            tri_sb = singles.tile([128, 128], BF16, name="tri_sb")
            nc.sync.dma_start(out=tri_sb, in_=tri[:, :])
            ones128 = singles.tile([128, 1], BF16, name="ones128")
            nc.vector.memset(ones128, 1.0)

            x = [None] * NT

            def load_x2(ta, tb):
                """Load two x tiles with ci-interleaved sub-DMAs so the first
                contraction chunks of both tiles land early."""
                for tt in (ta, tb):
                    x[tt] = xp.tile([128, KC, 512], BF16, tag="x", name=f"x{tt}")
                for ci in range(4):
                    for tt in (ta, tb):
                        nc.sync.dma_start(
                            out=x[tt][:, ds(8 * ci, 8), :],
                            in_=xT[ds(1024 * ci, 1024), ts(tt, 512)].rearrange(
                                "(c p) t -> p c t", p=128),
                        )

            kt = [None] * NT          # kt[tt]: [128, 8, 512] bf16 (dim-major)
            qt = [None] * NT          # qt[tt]: same shape, pool-rotated (bufs=2)
            vt = [None] * (NT * 4)    # vt[j]: [128, 1024] bf16 (token-major)
            atiles = {}               # (tt, b) -> [128, 512] bf16 attn out

            PTAGS = ("pa", "pb", "pc", "pd")

            def rope_evac(ps, dst, tt, par):
                """Evacuate psum [128,512] (rows 0-31 rot-even, 32-63 rot-odd,
                64-127 pass) into bf16 dst view, applying the GPT-J rotation.
                Multiplies read PSUM (exempt from the SB/SB same-base-partition
                rule); the final sub/add see partition-aligned SBUF operands."""
                tcols = ts(tt, 512)
                nc.scalar.activation(out=dst[64:128, :], in_=ps[64:128, :], func=Copy)
                tc_eo = rtp.tile([64, 512], BF16, tag="rt", name="tc_eo")
                nc.vector.tensor_mul(out=tc_eo, in0=ps[0:64, :], in1=trig_sb[0:64, tcols])
                ts_eo = rtp.tile([64, 512], BF16, tag="rt", name="ts_eo")
                nc.vector.tensor_mul(out=ts_eo[0:32, :], in0=ps[32:64, :], in1=trig_sb[64:96, tcols])
                nc.vector.tensor_mul(out=ts_eo[32:64, :], in0=ps[0:32, :], in1=trig_sb[96:128, tcols])
                nc.gpsimd.tensor_sub(out=dst[0:32, :], in0=tc_eo[0:32, :], in1=ts_eo[0:32, :])
                nc.gpsimd.tensor_add(out=dst[32:64, :], in0=tc_eo[32:64, :], in1=ts_eo[32:64, :])

            def proj_pass(half, wr, qr, is_q, first=False):
                """ob-pair (head qr) q/k projection over this half's 1024 tokens.
                first=True: weight loads all on the scalar queue (sync is busy
                streaming x at kernel start)."""
                t0, t1 = 2 * half, 2 * half + 1
                ps = [psp.tile([128, 512], F32, tag=PTAGS[i], name=f"pp{i}")
                      for i in range(4)]
                for dc in range(KC):
                    w_t = wqk.tile([128, 256], BF16, tag="w", name="w_t")
                    eng = nc.scalar if first else (nc.sync if dc % 2 == 0 else nc.scalar)
                    eng.dma_start(out=w_t, in_=wr[qr, dc, :, :])
                    st, sp = dc == 0, dc == KC - 1
                    nc.tensor.matmul(ps[0], w_t[:, 0:128], x[t0][:, dc, :], start=st, stop=sp)
                    nc.tensor.matmul(ps[1], w_t[:, 0:128], x[t1][:, dc, :], start=st, stop=sp)
                    nc.tensor.matmul(ps[2], w_t[:, 128:256], x[t0][:, dc, :], start=st, stop=sp)
                    nc.tensor.matmul(ps[3], w_t[:, 128:256], x[t1][:, dc, :], start=st, stop=sp)
                for ti, tt in ((0, t0), (1, t1)):
                    dstbuf = qt[tt] if is_q else kt[tt]
                    rope_evac(ps[ti], dstbuf[:, 2 * qr, :], tt, ti)
                    eng = nc.scalar if ti == 0 else nc.vector
                    if ti == 0:
                        nc.scalar.activation(out=dstbuf[:, 2 * qr + 1, :], in_=ps[2 + ti], func=Copy)
                    else:
                        nc.vector.tensor_copy(dstbuf[:, 2 * qr + 1, :], ps[2 + ti])

            def v_group(half, vh):
                """v projection: 8 token-blocks (this half) x one dv-half of 512."""
                pv = [psp.tile([128, 512], F32, tag=PTAGS[i // 2], name=f"pv{i}")
                      for i in range(8)]
                for dc in range(KC):
                    wv_t = wvp.tile([128, 512], BF16, tag="wv", name="wv_t")
                    eng = nc.sync if dc % 2 == 0 else nc.scalar
                    eng.dma_start(out=wv_t, in_=wv_r[vh, dc, :, :])
                    st, sp = dc == 0, dc == KC - 1
                    for tb in range(8):
                        tt = 2 * half + tb // 4
                        nc.tensor.matmul(pv[tb], x[tt][:, dc, ds(128 * (tb % 4), 128)],
                                         wv_t, start=st, stop=sp)
                for tb in range(8):
                    j = 8 * half + tb
                    if vh == 0:
                        vt[j] = vp.tile([128, 1024], BF16, tag=f"v{j}", name=f"vt{j}")
                    if tb % 2 == 0:
                        nc.vector.tensor_copy(vt[j][:, ds(512 * vh, 512)], pv[tb])
                    else:
                        nc.scalar.activation(out=vt[j][:, ds(512 * vh, 512)], in_=pv[tb], func=Copy)

            def proj_half(half):
                t0, t1 = 2 * half, 2 * half + 1
                for tt in (t0, t1):
                    kt[tt] = kp.tile([128, 8, 512], BF16, tag=f"k{tt}", name=f"kt{tt}")
                    qt[tt] = qp.tile([128, 8, 512], BF16, tag="q", name=f"qt{tt}")
                for wi, (wr, is_q) in enumerate(((wq_r, True), (wk_r, False))):
                    for qr in range(WQR):
                        proj_pass(half, wr, qr, is_q,
                                  first=(half == 0 and wi == 0 and qr == 0))
                for vh in range(2):
                    v_group(half, vh)

            def attn_tile(tt):
                njt = 4 * tt + 4
                tails = []

                def emit_sc(h, jt):
                    a = jt - 4 * tt
                    c0 = 128 * a if a > 0 else 0
                    ktt, kj = jt // 4, jt % 4
                    sc = psp.tile([128, 512], F32, tag="pd", name="sc")
                    nc.tensor.matmul(sc[:, c0:], kt[ktt][:, 2 * h, ds(128 * kj, 128)],
                                     qt[tt][:, 2 * h, c0:], start=True, stop=False)
                    nc.tensor.matmul(sc[:, c0:], kt[ktt][:, 2 * h + 1, ds(128 * kj, 128)],
                                     qt[tt][:, 2 * h + 1, c0:], start=False, stop=True)
                    et = etp.tile([128, 512], BF16, tag="et", name="et")
                    nc.scalar.activation(out=et[:, c0:], in_=sc[:, c0:], func=Exp,
                                         scale=INV_SCALE)
                    if a >= 0:
                        nc.vector.tensor_mul(out=et[:, ds(c0, 128)], in0=et[:, ds(c0, 128)],
                                             in1=tri_sb)
                    return et, c0

                for h in range(NH):
                    av0 = psp.tile([128, 512], F32, tag="pa", name="av0")
                    av1 = psp.tile([128, 512], F32, tag="pb", name="av1")
                    sums = psp.tile([1, 512], F32, tag="pc", name="sums")
                    pend = [emit_sc(h, 0)]
                    if njt > 1:
                        pend.append(emit_sc(h, 1))
                    for jt in range(njt):
                        if jt + 2 < njt:
                            pend.append(emit_sc(h, jt + 2))
                        et, c0 = pend.pop(0)
                        st, sp = jt == 0, jt == njt - 1
                        nc.tensor.matmul(sums[:, c0:], ones128, et[:, c0:], start=st, stop=sp)
                        nc.tensor.matmul(av0[:, c0:], vt[jt][:, ds(256 * h, 128)],
                                         et[:, c0:], start=st, stop=sp)
                        nc.tensor.matmul(av1[:, c0:], vt[jt][:, ds(256 * h + 128, 128)],
                                         et[:, c0:], start=st, stop=sp)
                        if jt == 2 and tails:
                            tails.pop(0)()

                    def tail(h=h, av0=av0, av1=av1, sums=sums):
                        rcp = rcpp.tile([1, 512], F32, tag="rcp", name="rcp")
                        scr = rcpp.tile([1, 512], F32, tag="scr", name="scr")
                        nc.vector.reciprocal_approx_accurate(out=rcp, in_=sums, scratch=scr)
                        bc = bcp.tile([128, 512], F32, tag="bc", name="bc")
                        nc.gpsimd.partition_broadcast(bc, rcp, channels=128)
                        a0 = atp.tile([128, 512], BF16, tag="at", name="a0")
                        a1 = atp.tile([128, 512], BF16, tag="at", name="a1")
                        nc.vector.tensor_mul(out=a0, in0=av0, in1=bc)
                        nc.vector.tensor_mul(out=a1, in0=av1, in1=bc)
                        atiles[(tt, 2 * h)], atiles[(tt, 2 * h + 1)] = a0, a1
                    tails.append(tail)
                while tails:
                    tails.pop(0)()

            def oproj_ctp(tt, ctp, ci, dest_rows):
                """One 256-out-dim column-pair of the out projection for tile tt.
                dest_rows: (tensor, row_offset) for the 256 output rows."""
                tga, tgb = (PTAGS[0], PTAGS[1]) if ci % 2 == 0 else (PTAGS[2], PTAGS[3])
                po0 = psp.tile([128, 512], F32, tag=tga, name="po0")
                po1 = psp.tile([128, 512], F32, tag=tgb, name="po1")
                for dc in range(QL // 128):
                    wo_t = wop.tile([128, 256], BF16, tag="wo", name="wo_t")
                    eng = nc.sync if dc % 2 == 0 else nc.scalar
                    eng.dma_start(out=wo_t, in_=wo_p[ctp, dc, :, :])
                    st, sp = dc == 0, dc == QL // 128 - 1
                    nc.tensor.matmul(po0, wo_t[:, 0:128], atiles[(tt, dc)], start=st, stop=sp)
                    nc.tensor.matmul(po1, wo_t[:, 128:256], atiles[(tt, dc)], start=st, stop=sp)
                dtensor, roff = dest_rows
                for hf, po in ((0, po0), (1, po1)):
                    o_sb = osbp.tile([128, 512], BF16, tag="o", name="o_sb")
                    if hf == 0:
                        nc.vector.tensor_copy(o_sb, po)
                    else:
                        nc.scalar.activation(out=o_sb, in_=po, func=Copy)
                    nc.sync.dma_start(out=dtensor[ds(roff + 128 * hf, 128), :], in_=o_sb)

            def oproj_tile(tt):
                for ci, ctp in enumerate(range(D // 256)):
                    oproj_ctp(tt, ctp, ci, (parts[tt], 256 * ctp))
                nc.gpsimd.collective_compute(
                    "ReduceScatter", mybir.AluOpType.add, replica_groups=GROUPS,
                    ins=[parts[tt][:]], outs=[rss[tt][:]],
                )
                # outc DMA deferred to program end: a sync-queue DMA that waits
                # on the collective would block all later weight loads (74us
                # PE stall measured at the proj_half(1) boundary).

            def oproj_tile_split(tt):
                # row-half i: per rank r the D-rows [1024r+512i, 1024r+512(i+1)),
                # i.e. ctp where (256*ctp % 1024)//512 == i.
                ccs = []
                for i in range(2):
                    ctps = [c for c in range(D // 256) if (256 * c % 1024) // 512 == i]
                    for ci, ctp in enumerate(ctps):
                        r = 256 * ctp // 1024
                        roff = 512 * r + (256 * ctp % 1024) - 512 * i
                        oproj_ctp(tt, ctp, ci, (parts2[i], roff))
                    ccs.append(nc.gpsimd.collective_compute(
                        "ReduceScatter", mybir.AluOpType.add, replica_groups=GROUPS,
                        ins=[parts2[i][:]], outs=[rss2[i][:]],
                    ))
                return ccs

            def flush_outc(split_tt, gate):
                # Pin these DMAs behind the first split-tile collective: a
                # sync-queue DMA waiting on a mid-kernel collective otherwise
                # gets scheduler-hoisted ahead of later weight loads and
                # stalls the whole queue (83us PE gap measured).
                for tt in (1, 0, 3):
                    d = nc.sync.dma_start(out=outc[:, ts(tt, 512)], in_=rss[tt][:])
                    add_dep_helper(d.ins, gate.ins, True, "outc flush after tail RS")
                for i in range(2):
                    nc.sync.dma_start(out=outc[ds(512 * i, 512), ts(split_tt, 512)],
                                      in_=rss2[i][:])

            # ---------------- schedule ----------------
            load_x2(0, 1)
            proj_half(0)
            load_x2(2, 3)
            attn_tile(1)
            oproj_tile(1)
            attn_tile(0)
            oproj_tile(0)
            proj_half(1)
            attn_tile(3)
            oproj_tile(3)
            attn_tile(2)
            split_ccs = oproj_tile_split(2)
            flush_outc(2, split_ccs[0])

    nc.compile()
    return nc


_ROT_PERM = np.concatenate([np.arange(0, ROT, 2), np.arange(1, ROT, 2), np.arange(ROT, HD)])


def make_in_maps(hidden_states, sin, cos, Wq, Wk, Wv, Wo):
    hidden_states = np.asarray(hidden_states, dtype=np.float32)
    sin = np.asarray(sin, dtype=np.float32)
    cos = np.asarray(cos, dtype=np.float32)
    Wq, Wk, Wv, Wo = (np.asarray(w, dtype=np.float32) for w in (Wq, Wk, Wv, Wo))

    ce = cos[0, :, 0, 0::2].T  # (32, S)
    co = cos[0, :, 0, 1::2].T
    se = sin[0, :, 0, 0::2].T
    so = sin[0, :, 0, 1::2].T
    trig = np.ascontiguousarray(np.concatenate([ce, co, se, so], axis=0)).astype(NPBF16)

    j = np.arange(128)[:, None]
    i = np.arange(128)[None, :]
    tri = (j <= i).astype(NPBF16)  # (128, 128) lower-left valid (k_off <= q_off)

    def pack_qk(w_loc_perm_T):
        # (D, QL) -> (WQR, KC, 128, 256)
        w4 = w_loc_perm_T.reshape(KC, 128, WQR, 256)
        return np.ascontiguousarray(w4.transpose(2, 0, 1, 3))

    def pack_v(wvT_loc):
        w4 = wvT_loc.reshape(KC, 128, 2, 512)
        return np.ascontiguousarray(w4.transpose(2, 0, 1, 3))

    in_maps = []
    for c in range(NCORES):
        dp, tp = divmod(c, TP)
        xT = np.ascontiguousarray(hidden_states[dp].T).astype(NPBF16)
        rows = np.arange(QL * tp, QL * (tp + 1))
        perm_rows = np.concatenate([QL * tp + 256 * h + _ROT_PERM for h in range(NH)])
        wqT = np.ascontiguousarray(Wq[perm_rows].T).astype(NPBF16)  # (D, QL)
        wkT = np.ascontiguousarray(Wk[perm_rows].T).astype(NPBF16)
        wvT = np.ascontiguousarray(Wv[rows].T).astype(NPBF16)
        woT = Wo[:, rows].T.astype(NPBF16)                          # (QL, D)
        wo_p = np.ascontiguousarray(
            woT.reshape(QL // 128, 128, D // 256, 256).transpose(2, 0, 1, 3))
        in_maps.append({
            "xT": xT,
            "wq_r": pack_qk(wqT),
            "wk_r": pack_qk(wkT),
            "wv_r": pack_v(wvT),
            "wo_p": wo_p,
            "trig": trig,
            "tri": tri,
        })
    return in_maps


def assemble_output(results):
    out = np.empty((B, S, D), dtype=np.float32)
    for dp in range(DP):
        outT = np.concatenate(
            [results[dp * TP + tp]["out_chunk"].astype(np.float32) for tp in range(TP)],
            axis=0,
        )  # (D, S)
        out[dp] = outT.T
    return out


_NC_CACHE = None


def _get_nc():
    global _NC_CACHE
    if _NC_CACHE is None:
        _NC_CACHE = build_nc()
    return _NC_CACHE


def run(trace=False, **inputs):
    in_maps = make_in_maps(**inputs)
    nc = _get_nc()
    r = run_bass_kernel_spmd(nc, in_maps, core_ids=list(range(NCORES)), trace=trace)
    return assemble_output(r.results), r


def kernel(**inputs):
    out, _ = run(trace=False, **inputs)
    return out


# revision 18
# speedup vs baseline: 1.0979x; 1.0722x over previous
"""GPT-J attention (B=2, S=2048, D=4096, H=16, HD=256, ROT=64, causal) on 8 TRN2 NeuronCores.

Sharding: DP over batch (2 groups of 4 cores) x TP over heads (4 heads/core).

v2 restructure vs the session-1 baseline:
- Weights stream once per 1024-token half (not once per 512-token tile):
  x / q / k / v are SBUF-resident, cutting weight DMA 134MB -> ~58MB and
  keeping the PE warm (no HAM oscillation from DMA stalls).
- Attention is software-pipelined 2 jt-blocks deep; softmax tails lag one
  head so the reciprocal never stalls the PE.
- gpsimd.partition_broadcast replaces the PE broadcast-matmul + copy;
  rope runs in bf16 with 2 of 5 elementwise ops on the idle GpSimd.
- Diagonal causal blocks compute only columns [128a:512] (column trim).
- Schedule: proj(half0) -> attn/outproj tiles 1,0 -> proj(half1) ->
  attn/outproj tiles 3,2; per-tile ReduceScatter overlaps later compute;
  the last tile's out-proj is split into two row-halves with staggered
  ReduceScatters to shrink the exposed tail.

Compute dtype: bf16 on the TensorEngine (f32 PSUM accumulation).
"""

import numpy as np
import ml_dtypes

import concourse.bass as bass
import concourse.mybir as mybir
import concourse.tile as tile
from concourse.tile import add_dep_helper
from concourse import bacc
from concourse.bass_utils import run_bass_kernel_spmd

B, S, D = 2, 2048, 4096
H, HD, ROT = 16, 256, 64
NCORES, DP, TP = 8, 2, 4
NH = H // TP              # 4 local heads
QL = NH * HD              # 1024 local q/k/v dims
T = S                     # local tokens (one batch per DP group)
NT = T // 512             # 4 token tiles of 512
KC = D // 128             # 32 contraction chunks
WQR = QL // 256           # 4 ob-pair passes for q/k (= one head each)
BF16 = mybir.dt.bfloat16
F32 = mybir.dt.float32
INV_SCALE = 1.0 / 16.0    # 1/sqrt(HD)
GROUPS = [[0, 1, 2, 3], [4, 5, 6, 7]]
NPBF16 = ml_dtypes.bfloat16
Copy = mybir.ActivationFunctionType.Copy
Exp = mybir.ActivationFunctionType.Exp
ds, ts = bass.ds, bass.ts


def build_nc():
    nc = bacc.Bacc("TRN2", target_bir_lowering=False, num_devices=NCORES)
    xT = nc.declare_dram_parameter("xT", [D, T], BF16, isOutput=False)
    wq_r = nc.declare_dram_parameter("wq_r", [WQR, KC, 128, 256], BF16, isOutput=False)
    wk_r = nc.declare_dram_parameter("wk_r", [WQR, KC, 128, 256], BF16, isOutput=False)
    wv_r = nc.declare_dram_parameter("wv_r", [2, KC, 128, 512], BF16, isOutput=False)
    wo_p = nc.declare_dram_parameter("wo_p", [D // 256, QL // 128, 128, 256], BF16, isOutput=False)
    # trig rows: [cos_even(32); cos_odd(32); sin_even(32); sin_odd(32)]
    trig = nc.declare_dram_parameter("trig", [128, T], BF16, isOutput=False)
    tri = nc.declare_dram_parameter("tri", [128, 128], BF16, isOutput=False)
    outc = nc.declare_dram_parameter("out_chunk", [QL, T], BF16, isOutput=True)

    # RS staging. Tiles 1, 0, 3 use full [D,512] buffers; tile 2 (the tail)
    # is split into two row-halves: parts2[i][r*512:(r+1)*512] holds rank r's
    # D-rows [1024r + 512i, 1024r + 512(i+1)).
    parts = {tt: nc.dram_tensor(f"part{tt}", [D, 512], BF16) for tt in (0, 1, 3)}
    rss = {tt: nc.dram_tensor(f"rs{tt}", [QL, 512], BF16) for tt in (0, 1, 3)}
    parts2 = [nc.dram_tensor(f"part2_{i}", [D // 2, 512], BF16) for i in range(2)]
    rss2 = [nc.dram_tensor(f"rs2_{i}", [QL // 2, 512], BF16) for i in range(2)]

    with tile.TileContext(nc) as tc:
        with (
            tc.tile_pool(name="singles", bufs=1) as singles,
            tc.tile_pool(name="xp", bufs=2) as xp,
            tc.tile_pool(name="kp", bufs=1) as kp,
            tc.tile_pool(name="qp", bufs=2) as qp,
            tc.tile_pool(name="vp", bufs=1) as vp,
            tc.tile_pool(name="wqk", bufs=10) as wqk,
            tc.tile_pool(name="wvp", bufs=6) as wvp,
            tc.tile_pool(name="wop", bufs=10) as wop,
            tc.tile_pool(name="etp", bufs=3) as etp,
            tc.tile_pool(name="rt", bufs=4) as rtp,
            tc.tile_pool(name="rcpp", bufs=2) as rcpp,
            tc.tile_pool(name="bcp", bufs=2) as bcp,
            tc.tile_pool(name="atp", bufs=10) as atp,
            tc.tile_pool(name="osb", bufs=4) as osbp,
            tc.tile_pool(name="ps", bufs=2, space="PSUM") as psp,
        ):
            # --- constants ---
            trig_sb = singles.tile([128, T], BF16, name="trig_sb")
            nc.sync.dma_start(out=trig_sb, in_=trig[:, :])
            tri_sb = singles.tile([128, 128], BF16, name="tri_sb")
            nc.sync.dma_start(out=tri_sb, in_=tri[:, :])
            ones128 = singles.tile([128, 1], BF16, name="ones128")
            nc.vector.memset(ones128, 1.0)

            x = [None] * NT

            def load_x2(ta, tb):
                """Load two x tiles with ci-interleaved sub-DMAs so the first
                contraction chunks of both tiles land early."""
                for tt in (ta, tb):
                    x[tt] = xp.tile([128, KC, 512], BF16, tag="x", name=f"x{tt}")
                for ci in range(4):
                    for tt in (ta, tb):
                        nc.sync.dma_start(
                            out=x[tt][:, ds(8 * ci, 8), :],
                            in_=xT[ds(1024 * ci, 1024), ts(tt, 512)].rearrange(
                                "(c p) t -> p c t", p=128),
                        )

            kt = [None] * NT          # kt[tt]: [128, 8, 512] bf16 (dim-major)
            qt = [None] * NT          # qt[tt]: same shape, pool-rotated (bufs=2)
            vt = [None] * (NT * 4)    # vt[j]: [128, 1024] bf16 (token-major)
            atiles = {}               # (tt, b) -> [128, 512] bf16 attn out

            PTAGS = ("pa", "pb", "pc", "pd")

            def rope_evac(ps, dst, tt, par):
                """Evacuate psum [128,512] (rows 0-31 rot-even, 32-63 rot-odd,
                64-127 pass) into bf16 dst view, applying the GPT-J rotation.
                Multiplies read PSUM (exempt from the SB/SB same-base-partition
                rule); the final sub/add see partition-aligned SBUF operands."""
                tcols = ts(tt, 512)
                nc.scalar.activation(out=dst[64:128, :], in_=ps[64:128, :], func=Copy)
                tc_eo = rtp.tile([64, 512], BF16, tag="rt", name="tc_eo")
                nc.vector.tensor_mul(out=tc_eo, in0=ps[0:64, :], in1=trig_sb[0:64, tcols])
                ts_eo = rtp.tile([64, 512], BF16, tag="rt", name="ts_eo")
                nc.vector.tensor_mul(out=ts_eo[0:32, :], in0=ps[32:64, :], in1=trig_sb[64:96, tcols])
                nc.vector.tensor_mul(out=ts_eo[32:64, :], in0=ps[0:32, :], in1=trig_sb[96:128, tcols])
                nc.gpsimd.tensor_sub(out=dst[0:32, :], in0=tc_eo[0:32, :], in1=ts_eo[0:32, :])
                nc.gpsimd.tensor_add(out=dst[32:64, :], in0=tc_eo[32:64, :], in1=ts_eo[32:64, :])

            def proj_pass(half, wr, qr, is_q, first=False):
                """ob-pair (head qr) q/k projection over this half's 1024 tokens.
                first=True: weight loads all on the scalar queue (sync is busy
                streaming x at kernel start)."""
                t0, t1 = 2 * half, 2 * half + 1
                ps = [psp.tile([128, 512], F32, tag=PTAGS[i], name=f"pp{i}")
                      for i in range(4)]
                last = None
                for dc in range(KC):
                    w_t = wqk.tile([128, 256], BF16, tag="w", name="w_t")
                    eng = nc.scalar if first else (nc.sync if dc % 2 == 0 else nc.scalar)
                    eng.dma_start(out=w_t, in_=wr[qr, dc, :, :])
                    st, sp = dc == 0, dc == KC - 1
                    nc.tensor.matmul(ps[0], w_t[:, 0:128], x[t0][:, dc, :], start=st, stop=sp)
                    nc.tensor.matmul(ps[1], w_t[:, 0:128], x[t1][:, dc, :], start=st, stop=sp)
                    nc.tensor.matmul(ps[2], w_t[:, 128:256], x[t0][:, dc, :], start=st, stop=sp)
                    last = nc.tensor.matmul(ps[3], w_t[:, 128:256], x[t1][:, dc, :], start=st, stop=sp)
                for ti, tt in ((0, t0), (1, t1)):
                    dstbuf = qt[tt] if is_q else kt[tt]
                    rope_evac(ps[ti], dstbuf[:, 2 * qr, :], tt, ti)
                    if ti == 0:
                        nc.scalar.activation(out=dstbuf[:, 2 * qr + 1, :], in_=ps[2 + ti], func=Copy)
                    else:
                        nc.vector.tensor_copy(dstbuf[:, 2 * qr + 1, :], ps[2 + ti])
                return last

            def v_group(half, vh):
                """v projection: 8 token-blocks (this half) x one dv-half of 512."""
                pv = [psp.tile([128, 512], F32, tag=PTAGS[i // 2], name=f"pv{i}")
                      for i in range(8)]
                for dc in range(KC):
                    wv_t = wvp.tile([128, 512], BF16, tag="wv", name="wv_t")
                    eng = nc.sync if dc % 2 == 0 else nc.scalar
                    eng.dma_start(out=wv_t, in_=wv_r[vh, dc, :, :])
                    st, sp = dc == 0, dc == KC - 1
                    for tb in range(8):
                        tt = 2 * half + tb // 4
                        nc.tensor.matmul(pv[tb], x[tt][:, dc, ds(128 * (tb % 4), 128)],
                                         wv_t, start=st, stop=sp)
                for tb in range(8):
                    j = 8 * half + tb
                    if vh == 0:
                        vt[j] = vp.tile([128, 1024], BF16, tag=f"v{j}", name=f"vt{j}")
                    if tb % 2 == 0:
                        nc.vector.tensor_copy(vt[j][:, ds(512 * vh, 512)], pv[tb])
                    else:
                        nc.scalar.activation(out=vt[j][:, ds(512 * vh, 512)], in_=pv[tb], func=Copy)

            def proj_half_qk(half):
                t0, t1 = 2 * half, 2 * half + 1
                for tt in (t0, t1):
                    kt[tt] = kp.tile([128, 8, 512], BF16, tag=f"k{tt}", name=f"kt{tt}")
                    qt[tt] = qp.tile([128, 8, 512], BF16, tag="q", name=f"qt{tt}")
                last = None
                for wi, (wr, is_q) in enumerate(((wq_r, True), (wk_r, False))):
                    for qr in range(WQR):
                        last = proj_pass(half, wr, qr, is_q,
                                         first=(half == 0 and wi == 0 and qr == 0))
                return last

            def proj_half_v(half):
                for vh in range(2):
                    v_group(half, vh)

            def attn_tile(tt):
                njt = 4 * tt + 4
                tails = []

                def emit_sc(h, jt):
                    a = jt - 4 * tt
                    c0 = 128 * a if a > 0 else 0
                    ktt, kj = jt // 4, jt % 4
                    sc = psp.tile([128, 512], F32, tag="pd", name="sc")
                    nc.tensor.matmul(sc[:, c0:], kt[ktt][:, 2 * h, ds(128 * kj, 128)],
                                     qt[tt][:, 2 * h, c0:], start=True, stop=False)
                    nc.tensor.matmul(sc[:, c0:], kt[ktt][:, 2 * h + 1, ds(128 * kj, 128)],
                                     qt[tt][:, 2 * h + 1, c0:], start=False, stop=True)
                    et = etp.tile([128, 512], BF16, tag="et", name="et")
                    nc.scalar.activation(out=et[:, c0:], in_=sc[:, c0:], func=Exp,
                                         scale=INV_SCALE)
                    if a >= 0:
                        nc.vector.tensor_mul(out=et[:, ds(c0, 128)], in0=et[:, ds(c0, 128)],
                                             in1=tri_sb)
                    return et, c0

                for h in range(NH):
                    av0 = psp.tile([128, 512], F32, tag="pa", name="av0")
                    av1 = psp.tile([128, 512], F32, tag="pb", name="av1")
                    sums = psp.tile([1, 512], F32, tag="pc", name="sums")
                    pend = [emit_sc(h, 0)]
                    if njt > 1:
                        pend.append(emit_sc(h, 1))
                    for jt in range(njt):
                        if jt + 2 < njt:
                            pend.append(emit_sc(h, jt + 2))
                        et, c0 = pend.pop(0)
                        st, sp = jt == 0, jt == njt - 1
                        nc.tensor.matmul(sums[:, c0:], ones128, et[:, c0:], start=st, stop=sp)
                        nc.tensor.matmul(av0[:, c0:], vt[jt][:, ds(256 * h, 128)],
                                         et[:, c0:], start=st, stop=sp)
                        nc.tensor.matmul(av1[:, c0:], vt[jt][:, ds(256 * h + 128, 128)],
                                         et[:, c0:], start=st, stop=sp)
                        if jt == 2 and tails:
                            tails.pop(0)()

                    def tail(h=h, av0=av0, av1=av1, sums=sums):
                        rcp = rcpp.tile([1, 512], F32, tag="rcp", name="rcp")
                        scr = rcpp.tile([1, 512], F32, tag="scr", name="scr")
                        nc.vector.reciprocal_approx_accurate(out=rcp, in_=sums, scratch=scr)
                        bc = bcp.tile([128, 512], F32, tag="bc", name="bc")
                        nc.gpsimd.partition_broadcast(bc, rcp, channels=128)
                        a0 = atp.tile([128, 512], BF16, tag="at", name="a0")
                        a1 = atp.tile([128, 512], BF16, tag="at", name="a1")
                        nc.vector.tensor_mul(out=a0, in0=av0, in1=bc)
                        nc.vector.tensor_mul(out=a1, in0=av1, in1=bc)
                        atiles[(tt, 2 * h)], atiles[(tt, 2 * h + 1)] = a0, a1
                    tails.append(tail)
                while tails:
                    tails.pop(0)()

            def oproj_ctp(tt, ctp, ci, dest_rows):
                """One 256-out-dim column-pair of the out projection for tile tt.
                dest_rows: (tensor, row_offset) for the 256 output rows."""
                tga, tgb = (PTAGS[0], PTAGS[1]) if ci % 2 == 0 else (PTAGS[2], PTAGS[3])
                po0 = psp.tile([128, 512], F32, tag=tga, name="po0")
                po1 = psp.tile([128, 512], F32, tag=tgb, name="po1")
                for dc in range(QL // 128):
                    wo_t = wop.tile([128, 256], BF16, tag="wo", name="wo_t")
                    eng = nc.sync if dc % 2 == 0 else nc.scalar
                    eng.dma_start(out=wo_t, in_=wo_p[ctp, dc, :, :])
                    st, sp = dc == 0, dc == QL // 128 - 1
                    nc.tensor.matmul(po0, wo_t[:, 0:128], atiles[(tt, dc)], start=st, stop=sp)
                    nc.tensor.matmul(po1, wo_t[:, 128:256], atiles[(tt, dc)], start=st, stop=sp)
                dtensor, roff = dest_rows
                last = None
                for hf, po in ((0, po0), (1, po1)):
                    o_sb = osbp.tile([128, 512], BF16, tag="o", name="o_sb")
                    if hf == 0:
                        nc.vector.tensor_copy(o_sb, po)
                    else:
                        nc.scalar.activation(out=o_sb, in_=po, func=Copy)
                    last = nc.sync.dma_start(out=dtensor[ds(roff + 128 * hf, 128), :], in_=o_sb)
                return last

            def rs_tile(tt, gate=None):
                cc = nc.gpsimd.collective_compute(
                    "ReduceScatter", mybir.AluOpType.add, replica_groups=GROUPS,
                    ins=[parts[tt][:]], outs=[rss[tt][:]],
                )
                if gate is not None:
                    add_dep_helper(cc.ins, gate.ins, True, "RS delayed into DMA-quiet window")
                return cc

            def oproj_tile(tt):
                for ci, ctp in enumerate(range(D // 256)):
                    oproj_ctp(tt, ctp, ci, (parts[tt], 256 * ctp))

            def oproj_tile_split(tt):
                # row-half i: per rank r the D-rows [1024r+512i, 1024r+512(i+1)),
                # i.e. ctp where (256*ctp % 1024)//512 == i.
                ccs, last = [], None
                for i in range(2):
                    ctps = [c for c in range(D // 256) if (256 * c % 1024) // 512 == i]
                    for ci, ctp in enumerate(ctps):
                        r = 256 * ctp // 1024
                        roff = 512 * r + (256 * ctp % 1024) - 512 * i
                        last = oproj_ctp(tt, ctp, ci, (parts2[i], roff))
                    ccs.append(nc.gpsimd.collective_compute(
                        "ReduceScatter", mybir.AluOpType.add, replica_groups=GROUPS,
                        ins=[parts2[i][:]], outs=[rss2[i][:]],
                    ))
                return ccs, last

            def flush_outc(split_tt, gate_full, gate_split):
                # Gate placement so no sync-queue DMA ever sits waiting on an
                # in-flight collective ahead of later weight loads (the
                # scheduler otherwise hoists these and stalls the queue).
                for tt in (1, 0, 3):
                    d = nc.sync.dma_start(out=outc[:, ts(tt, 512)], in_=rss[tt][:])
                    add_dep_helper(d.ins, gate_full.ins, True, "outc flush after RS(3)")
                d = nc.sync.dma_start(out=outc[ds(0, 512), ts(split_tt, 512)], in_=rss2[0][:])
                add_dep_helper(d.ins, gate_split.ins, True, "outc2a after last split store")
                nc.sync.dma_start(out=outc[ds(512, 512), ts(split_tt, 512)], in_=rss2[1][:])

            # ---------------- schedule ----------------
            load_x2(0, 1)
            proj_half_qk(0)
            proj_half_v(0)
            load_x2(2, 3)
            attn_tile(1)
            oproj_tile(1)
            attn_tile(0)
            oproj_tile(0)
            last_qk = proj_half_qk(1)
            # RS(1)/RS(0) delayed into the v-group + attn(3) window: a 4MB RS
            # takes ~60-75us at the collective's ~54GB/s and starves model-DMA
            # weight loads while it runs; this window has little weight traffic.
            rs_tile(1, gate=last_qk)
            rs_tile(0)
            proj_half_v(1)
            attn_tile(3)
            oproj_tile(3)
            cc3 = rs_tile(3)
            attn_tile(2)
            split_ccs, last_split = oproj_tile_split(2)
            flush_outc(2, cc3, last_split)

    nc.compile()
    return nc


_ROT_PERM = np.concatenate([np.arange(0, ROT, 2), np.arange(1, ROT, 2), np.arange(ROT, HD)])


def make_in_maps(hidden_states, sin, cos, Wq, Wk, Wv, Wo):
    hidden_states = np.asarray(hidden_states, dtype=np.float32)
    sin = np.asarray(sin, dtype=np.float32)
    cos = np.asarray(cos, dtype=np.float32)
    Wq, Wk, Wv, Wo = (np.asarray(w, dtype=np.float32) for w in (Wq, Wk, Wv, Wo))

    ce = cos[0, :, 0, 0::2].T  # (32, S)
    co = cos[0, :, 0, 1::2].T
    se = sin[0, :, 0, 0::2].T
    so = sin[0, :, 0, 1::2].T
    trig = np.ascontiguousarray(np.concatenate([ce, co, se, so], axis=0)).astype(NPBF16)

    j = np.arange(128)[:, None]
    i = np.arange(128)[None, :]
    tri = (j <= i).astype(NPBF16)  # (128, 128) lower-left valid (k_off <= q_off)

    def pack_qk(w_loc_perm_T):
        # (D, QL) -> (WQR, KC, 128, 256)
        w4 = w_loc_perm_T.reshape(KC, 128, WQR, 256)
        return np.ascontiguousarray(w4.transpose(2, 0, 1, 3))

    def pack_v(wvT_loc):
        w4 = wvT_loc.reshape(KC, 128, 2, 512)
        return np.ascontiguousarray(w4.transpose(2, 0, 1, 3))

    in_maps = []
    for c in range(NCORES):
        dp, tp = divmod(c, TP)
        xT = np.ascontiguousarray(hidden_states[dp].T).astype(NPBF16)
        rows = np.arange(QL * tp, QL * (tp + 1))
        perm_rows = np.concatenate([QL * tp + 256 * h + _ROT_PERM for h in range(NH)])
        wqT = np.ascontiguousarray(Wq[perm_rows].T).astype(NPBF16)  # (D, QL)
        wkT = np.ascontiguousarray(Wk[perm_rows].T).astype(NPBF16)
        wvT = np.ascontiguousarray(Wv[rows].T).astype(NPBF16)
        woT = Wo[:, rows].T.astype(NPBF16)                          # (QL, D)
        wo_p = np.ascontiguousarray(
            woT.reshape(QL // 128, 128, D // 256, 256).transpose(2, 0, 1, 3))
        in_maps.append({
            "xT": xT,
            "wq_r": pack_qk(wqT),
            "wk_r": pack_qk(wkT),
            "wv_r": pack_v(wvT),
            "wo_p": wo_p,
            "trig": trig,
            "tri": tri,
        })
    return in_maps


def assemble_output(results):
    out = np.empty((B, S, D), dtype=np.float32)
    for dp in range(DP):
        outT = np.concatenate(
            [results[dp * TP + tp]["out_chunk"].astype(np.float32) for tp in range(TP)],
            axis=0,
        )  # (D, S)
        out[dp] = outT.T
    return out


_NC_CACHE = None


def _get_nc():
    global _NC_CACHE
    if _NC_CACHE is None:
        _NC_CACHE = build_nc()
    return _NC_CACHE


def run(trace=False, **inputs):
    in_maps = make_in_maps(**inputs)
    nc = _get_nc()
    r = run_bass_kernel_spmd(nc, in_maps, core_ids=list(range(NCORES)), trace=trace)
    return assemble_output(r.results), r


def kernel(**inputs):
    out, _ = run(trace=False, **inputs)
    return out
